# revision 10
# baseline (speedup 1.0000x reference)
"""Trainium2 Bass kernel for nn_DEFNet: 16-branch 1D conv (k=3..33) + bias + ReLU
+ channel-mean over x[32, 1, 262144] -> out[32, 262144].

Strategy (per core, 8 cores, 4 batch rows each):
  - Host builds a transposed sliding-window view xwinT[k, t] = xpad[64t + k]
    (k in [0,96)), a constant-ones row 96 (bias), zero rows to 112. Each
    channel-pair's conv+bias is matmuls into one [128, 1024] PSUM tile:
       psum[(c,p), t] = sum_k lhsT[k, 64c+p] * xwinT[k, t]
    with the /16 channel-mean folded into lhsT.
  - PSUM can only be drained by ScalarE and VectorE (GpSimd has no PSUM
    port), so the 8 pair-psums per block are split: scalar relus 3.5 of
    them (r0, rS0-2 + the low half of pair 5), vector chains 3 via fused
    relu+add STT onto r0 (-> otV partial) plus the high half of pair 5.
    GpSimd (which can't see PSUM but can stream SBUF) folds the three
    scalar relus with fp16 tensor_tensor adds -> otP partial.  Out per
    block: otV, otP, and the two unfolded half-tiles of pair 5 -- three
    fp16 partial-columns total.  Host: sum partials, fold the two 64-row
    halves, transpose to natural order.
  - Fill order interleaves the consumers so each of the 4 PSUM buffers is
    drained before its slot comes up again: [s:r0, s:rS0, d:a1, s:rS1,
    d:a2, split s/d, d:otV, s:rS2].
"""

import os

import numpy as np

import concourse.bass as bass
import concourse.mybir as mybir
import concourse.tile as tile
from concourse import bacc, bass_utils
from concourse.tile import TileContext

B, L = 32, 262144
NCONV, MAXK = 16, 33
NCORES = 8
ROWS = B // NCORES          # batch rows per core
P = 64                      # output positions per segment
W = 112                     # window rows (96 data + bias row 96 + zero pad)
HALO = 16
T = L // P                  # segments per row (4096)

# --- tunables -------------------------------------------------------------
BLK = 1024                  # segments per block (one pair-psum = 2 banks)
MMN = 512                   # matmul N cap
XBLK = 2048                 # segments per x-in DMA (2 blocks)
DT_X = mybir.dt.float16
DT_W = mybir.dt.float16
DT_E = mybir.dt.float16     # relu/accumulate dtype
F32 = mybir.dt.float32


def _support_mask():
    m = np.zeros((NCONV, MAXK), dtype=np.float32)
    c = MAXK // 2
    for i in range(1, NCONV + 1):
        m[i - 1, c - i:c + i + 1] = 1.0
    return m


def _build_lhsT(w, b):
    """[112, 8*128] f32; pair j cols j*128..(j+1)*128,
    lhsT[k, 64c+p] = wm[2j+c, k-p]/16 (k<96), lhsT[96, ...] = b/16."""
    wm = (np.asarray(w, np.float32) * _support_mask()) / 16.0
    bs = np.asarray(b, np.float32) / 16.0
    lhsT = np.zeros((W, 8 * 128), dtype=np.float32)
    for j in range(8):
        for c in range(2):
            ch = 2 * j + c
            for p in range(P):
                lhsT[p:p + MAXK, j * 128 + c * 64 + p] = wm[ch]
            lhsT[96, j * 128 + c * 64:j * 128 + c * 64 + P] = bs[ch]
    return lhsT


def _build_nc():
    nc = bacc.Bacc(
        "TRN2",
        target_bir_lowering=False,
        debug=False,
        enable_asserts=False,
        num_devices=NCORES,
    )
    xwin = nc.dram_tensor("xwin", [ROWS * W, T], DT_X, kind="ExternalInput").ap()
    wts = nc.dram_tensor("wts", [W, 8 * 128], DT_W, kind="ExternalInput").ap()
    # 3 partial-columns per block + 1 spare column for the last block's
    # unfolded rS2 (pool fold skipped there to shorten the drain tail)
    outH = nc.dram_tensor(
        "outH", [ROWS * 128, 3 * T + BLK], DT_E, kind="ExternalOutput").ap()

    relu = mybir.ActivationFunctionType.Relu
    op_max, op_add = mybir.AluOpType.max, mybir.AluOpType.add

    with TileContext(nc) as tc:
        with (
            tc.tile_pool(name="consts", bufs=1) as cpool,
            tc.tile_pool(name="xin", bufs=3) as xpool,
            tc.tile_pool(name="psum", bufs=4, space="PSUM") as pspool,
            tc.tile_pool(name="seed", bufs=3) as spool,
            tc.tile_pool(name="racc", bufs=3) as rpool,
            tc.tile_pool(name="acc", bufs=4) as apool,
            tc.tile_pool(name="fold", bufs=3) as fpool,
            tc.tile_pool(name="out", bufs=4) as opool,
        ):
            # prefetch the first x tile before anything else: it is the
            # long pole of the startup critical path
            x_first = xpool.tile([W, XBLK], DT_X, tag="xin")
            nc.sync.dma_start(x_first[:], xwin[0:W, 0:XBLK])

            w_sb = cpool.tile([W, 8 * 128], DT_W)
            nc.sync.dma_start(w_sb[:], wts[:])
            # warm scalar/vector/gpsimd views of w_sb so later ops carry
            # fewer distinct sync waits per instruction
            warm = cpool.tile([W, 8], DT_W)
            nc.vector.tensor_copy(out=warm[:], in_=w_sb[:, 0:8])
            warm3 = cpool.tile([W, 8], DT_W)
            nc.scalar.copy(warm3[:], w_sb[:, 0:8])
            warm4 = cpool.tile([W, 8], DT_W)
            nc.gpsimd.tensor_copy(out=warm4[:], in_=w_sb[:, 0:8])

            # fill order: position -> (pair j, consumer)
            # 0: scalar r0 (seed of the vector chain)
            # 1: scalar rS0
            # 2: vector a1 = relu(ps)+r0
            # 3: scalar rS1
            # 4: vector a2 = relu(ps)+a1
            # 5: split: scalar relu low half, vector relu high half
            # 6: vector otV = relu(ps)+a2
            # 7: scalar rS2
            for r in range(ROWS):
                for xb in range(T // XBLK):
                    if r == 0 and xb == 0:
                        x_sb = x_first
                    else:
                        x_sb = xpool.tile([W, XBLK], DT_X, tag="xin")
                        nc.sync.dma_start(
                            x_sb[:], xwin[r * W:(r + 1) * W,
                                          xb * XBLK:(xb + 1) * XBLK])
                    for sub in range(XBLK // BLK):
                        blk = xb * (XBLK // BLK) + sub
                        s0 = blk * BLK
                        xs = x_sb[:, sub * BLK:(sub + 1) * BLK]
                        last = (r == ROWS - 1) and (blk == T // BLK - 1)

                        r0 = spool.tile([128, BLK], DT_E, tag="r0")
                        rS0 = spool.tile([128, BLK], DT_E, tag="rS0")
                        rS1 = spool.tile([128, BLK], DT_E, tag="rS1")
                        rS2 = spool.tile([128, BLK], DT_E, tag="rS2")
                        a1 = apool.tile([128, BLK], DT_E, tag="a1")
                        a2 = apool.tile([128, BLK], DT_E, tag="a2")
                        f1 = fpool.tile([128, BLK], DT_E, tag="f1")
                        otV = opool.tile([128, BLK], DT_E, tag="otV")
                        otSs = opool.tile([128, BLK // 2], DT_E, tag="otSs")
                        otSv = opool.tile([128, BLK // 2], DT_E, tag="otSv")
                        otP = fpool.tile([128, BLK], DT_E, tag="otP")

                        def mm(j):
                            ps = pspool.tile([128, BLK], F32)
                            lhsT = w_sb[:, j * 128:(j + 1) * 128]
                            for m in range(BLK // MMN):
                                nc.tensor.matmul(
                                    ps[:, m * MMN:(m + 1) * MMN], lhsT,
                                    xs[:, m * MMN:(m + 1) * MMN],
                                    start=True, stop=True)
                            return ps

                        # fill 0: scalar seed r0
                        ps = mm(0)
                        nc.scalar.activation(r0[:], ps[:], relu)
                        # fill 1: scalar rS0
                        ps = mm(1)
                        nc.scalar.activation(rS0[:], ps[:], relu)
                        # fill 2: vector a1
                        ps = mm(2)
                        nc.vector.scalar_tensor_tensor(
                            a1[:], ps[:], 0.0, r0[:], op_max, op_add)
                        # fill 3: scalar rS1
                        ps = mm(3)
                        nc.scalar.activation(rS1[:], ps[:], relu)
                        # fill 4: vector a2
                        ps = mm(4)
                        nc.vector.scalar_tensor_tensor(
                            a2[:], ps[:], 0.0, a1[:], op_max, op_add)
                        # fill 5: split drain -> two half tiles (unfolded)
                        ps = mm(5)
                        nc.scalar.activation(
                            otSs[:], ps[:, 0:BLK // 2], relu)
                        nc.vector.tensor_scalar(
                            otSv[:], ps[:, BLK // 2:BLK],
                            scalar1=0.0, scalar2=None, op0=op_max)
                        # fill 6: vector otV
                        ps = mm(6)
                        nc.vector.scalar_tensor_tensor(
                            otV[:], ps[:], 0.0, a2[:], op_max, op_add)
                        # fill 7: scalar rS2
                        ps = mm(7)
                        nc.scalar.activation(rS2[:], ps[:], relu)

                        # gpsimd folds the scalar relus in SBUF (fp16)
                        nc.gpsimd.tensor_tensor(f1[:], rS0[:], rS1[:], op_add)
                        c0 = 3 * s0
                        if last:
                            # skip the second fold to shorten the tail;
                            # ship f1 and rS2 separately
                            nc.sync.dma_start(
                                outH[r * 128:(r + 1) * 128,
                                     c0 + BLK:c0 + 2 * BLK], f1[:])
                            nc.sync.dma_start(
                                outH[r * 128:(r + 1) * 128,
                                     3 * T:3 * T + BLK], rS2[:])
                        else:
                            nc.gpsimd.tensor_tensor(
                                otP[:], f1[:], rS2[:], op_add)
                            nc.sync.dma_start(
                                outH[r * 128:(r + 1) * 128,
                                     c0 + BLK:c0 + 2 * BLK], otP[:])
                        nc.sync.dma_start(
                            outH[r * 128:(r + 1) * 128, c0:c0 + BLK], otV[:])
                        nc.sync.dma_start(
                            outH[r * 128:(r + 1) * 128,
                                 c0 + 2 * BLK:c0 + 2 * BLK + BLK // 2],
                            otSs[:])
                        nc.sync.dma_start(
                            outH[r * 128:(r + 1) * 128,
                                 c0 + 2 * BLK + BLK // 2:c0 + 3 * BLK],
                            otSv[:])
    nc.compile()
    return nc


_NC_CACHE = None


def _get_nc():
    global _NC_CACHE
    if _NC_CACHE is None:
        _NC_CACHE = _build_nc()
    return _NC_CACHE


LAST_RESULTS = None


def _install_ntff_hook():
    """Provide antenv.axon_hooks (absent on this image) so
    run_bass_kernel_spmd(trace=True) can capture NTFF profiles via the
    axon PJRT plugin's C ABI. Also stub the artifact upload (no bucket
    creds in-container)."""
    import contextlib
    import ctypes
    import sys
    import types

    try:
        from antenv.axon_hooks import get_axon_ntff_profile_hook  # noqa: F401
        return  # real module present
    except ImportError:
        pass

    so_path = "/opt/axon/libaxon_pjrt.so"
    lib = ctypes.CDLL(so_path)
    lib.axon_start_nrt_profile.argtypes = [
        ctypes.POINTER(ctypes.c_int64), ctypes.c_size_t]
    lib.axon_start_nrt_profile.restype = ctypes.c_int64
    lib.axon_stop_nrt_profile.argtypes = [ctypes.c_char_p]
    lib.axon_stop_nrt_profile.restype = ctypes.c_int64

    @contextlib.contextmanager
    def _hook(output_dir, device_ids):
        import jax
        jax.devices()
        if device_ids:
            ids = (ctypes.c_int64 * len(device_ids))(*device_ids)
            rc = lib.axon_start_nrt_profile(ids, len(device_ids))
        else:
            rc = lib.axon_start_nrt_profile(None, 0)
        if rc != 0:
            raise RuntimeError(f"axon_start_nrt_profile rc={rc}")
        try:
            yield
        finally:
            n = lib.axon_stop_nrt_profile(str(output_dir).encode())
            print(f"ntff profile: {n} file(s) -> {output_dir}")

    mod = types.ModuleType("antenv.axon_hooks")
    mod.get_axon_ntff_profile_hook = lambda: _hook
    mod.set_axon_ntff_profile_hook = lambda h: None
    sys.modules["antenv.axon_hooks"] = mod
    bass_utils.upload_artifacts = lambda tmpdir: f"file://{tmpdir}"


def host_inputs(x, w, b):
    """Build the 8 per-core input maps from the full problem inputs."""
    x = np.asarray(x, np.float32)
    xpad = np.pad(x[:, 0, :], ((0, 0), (HALO, HALO)))  # [B, L+32]
    s = xpad.strides
    np_x = mybir.dt.np(DT_X)
    xwinT = np.lib.stride_tricks.as_strided(
        xpad, shape=(B, 96, T), strides=(s[0], s[1], P * s[1]))

    lhsT = _build_lhsT(w, b).astype(mybir.dt.np(DT_W))

    in_maps = []
    for core in range(NCORES):
        rows = xwinT[core * ROWS:(core + 1) * ROWS]          # [4, 96, T]
        xw = np.zeros((ROWS, W, T), dtype=np_x)
        xw[:, :96, :] = rows
        xw[:, 96, :] = 1.0                                   # bias row
        in_maps.append({
            "xwin": xw.reshape(ROWS * W, T),
            "wts": lhsT,
        })
    return in_maps


def kernel(x, w, b):
    global LAST_RESULTS
    in_maps = host_inputs(x, w, b)
    nc = _get_nc()
    trace = bool(os.environ.get("KERNEL_TRACE"))
    if trace:
        _install_ntff_hook()
    res = bass_utils.run_bass_kernel_spmd(
        nc, in_maps, core_ids=list(range(NCORES)), trace=trace,
        **({"trace_cores": [0]} if trace else {}),
    )
    LAST_RESULTS = res

    out = np.empty((B, L), dtype=np.float32)
    for core in range(NCORES):
        oH = res.results[core]["outH"].astype(np.float32)    # [512, 3T+BLK]
        main = oH[:, :3 * T].reshape(ROWS, 128, T // BLK, 3, BLK)
        acc = main.sum(axis=3)                               # [ROWS,128,nb,BLK]
        # last block of the last row shipped rS2 unfolded in the spare col
        acc[ROWS - 1, :, T // BLK - 1, :] += oH[(ROWS - 1) * 128:, 3 * T:]
        acc = acc.reshape(ROWS, 2, P, T)
        folded = acc.sum(axis=1)                             # [ROWS, P, T]
        for r in range(ROWS):
            # position = t*64 + p  ->  transpose to [T, P]
            out[core * ROWS + r] = folded[r].T.reshape(L)
    return out
